# revision 5
# baseline (speedup 1.0000x reference)
"""Trainium2 Bass kernel for nn_DictionaryLearning (batch OMP / vq_codebook).

Strategy (data-parallel over the flattened sample axis, per sharding hint):
- Host: z_e (B,C,H,W) -> channels-last -> raw reshape X (64, 131072).
  Shard columns across 8 cores (16384 each). Dictionary replicated.
- Device per core: for each 128-column subtile run 5 OMP iterations:
    corr   = matmul(lhsT=residual(64,128), rhs=Dn(64,512)) -> PSUM (128 cols, 512 atoms)
    corr2  = square(corr)                 [ScalarE, PSUM->SBUF]
    m2     = reduce_max(corr2)            [VectorE]
    num    = accum[(corr2==m2)*corr]      [VectorE scalar_tensor_tensor, signed winner]
    idx    = accum[(corr2==m2)*iota]      [VectorE scalar_tensor_tensor]
    d_sel  = indirect-DMA gather of [DnT | -s] rows by idx (batched per group)
    -alpha = num / (-s[idx])              [VectorE divide]
    resT  -= alpha*d_selT                 [VectorE fused mult-add, T-layout]
    res    = transpose(resT)              [TensorE + ScalarE evac] for next matmul
- Outputs: final residual (64, cols) + per-subtile (idx, num) pairs.
  Host reconstructs z_out = X - residual, loss = 1.25*mean(res^2),
  coeffs scattered sparse -> dense with alpha = num/(s[idx]+eps).
"""
import sys
for _p in ('/opt/trn_rl_repo', '/root/.axon_site/_ro/trn_rl_repo'):
    if _p not in sys.path:
        sys.path.insert(0, _p)
from contextlib import ExitStack

import numpy as np

import concourse.bass as bass
import concourse.bacc as bacc
import concourse.tile as tile
from concourse import mybir
from concourse.bass_utils import run_bass_kernel_spmd
from concourse.masks import make_identity

F32 = mybir.dt.float32
EMBED = 64
NATOMS = 512
SPAR = 5
NCORES = 8
TOTAL_COLS = 32 * 64 * 64  # 131072
SUB = 128                  # columns per subtile (matmul out partitions)
EPS = 1e-10


def build_nc(cols_per_core: int, group: int):
    """Build + compile the per-core SPMD bass module."""
    nsub = cols_per_core // SUB
    ngroups = nsub // group
    assert nsub * SUB == cols_per_core and ngroups * group == nsub

    nc = bacc.Bacc("TRN2", target_bir_lowering=False, debug=False)

    x_d = nc.dram_tensor("x", [EMBED, cols_per_core], F32, kind="ExternalInput")
    dn_d = nc.dram_tensor("dn", [EMBED, NATOMS], F32, kind="ExternalInput")
    cn_d = nc.dram_tensor("constn", [128, NATOMS], F32, kind="ExternalInput")
    tab_d = nc.dram_tensor("tab", [NATOMS, EMBED + 1], F32, kind="ExternalInput")

    res_d = nc.dram_tensor("res", [EMBED, cols_per_core], F32, kind="ExternalOutput")
    sc_d = nc.dram_tensor("scan", [ngroups * 128, group * 2 * SPAR], F32,
                          kind="ExternalOutput")

    AX = mybir.AxisListType.X
    OP = mybir.AluOpType

    with tile.TileContext(nc) as tc, ExitStack() as ctx:
        const = ctx.enter_context(tc.tile_pool(name="const", bufs=1))
        xgp = ctx.enter_context(tc.tile_pool(name="xg", bufs=2))
        rgo = ctx.enter_context(tc.tile_pool(name="resgo", bufs=2))
        c2p = ctx.enter_context(tc.tile_pool(name="c2", bufs=3))
        wp = ctx.enter_context(tc.tile_pool(name="w", bufs=3))
        rTp = ctx.enter_context(tc.tile_pool(name="rT", bufs=2 * group + 2))
        rsp = ctx.enter_context(tc.tile_pool(name="rs", bufs=group + 2))
        dsp = ctx.enter_context(tc.tile_pool(name="dsel", bufs=2))
        stp = ctx.enter_context(tc.tile_pool(name="stage", bufs=2))
        ixp = ctx.enter_context(tc.tile_pool(name="idxg", bufs=2))
        tiny = ctx.enter_context(tc.tile_pool(name="tiny", bufs=12))
        cps = ctx.enter_context(tc.tile_pool(name="cpsum", bufs=3, space="PSUM"))
        tps = ctx.enter_context(tc.tile_pool(name="tpsum", bufs=4, space="PSUM"))

        dn_sb = const.tile([EMBED, NATOMS], F32)
        nc.sync.dma_start(dn_sb[:], dn_d[:])
        cn_sb = const.tile([128, NATOMS], F32)
        nc.sync.dma_start(cn_sb[:], cn_d[:])
        i64 = const.tile([64, 64], F32)
        make_identity(nc, i64[:])
        i128 = const.tile([128, 128], F32)
        make_identity(nc, i128[:])

        for g in range(ngroups):
            xg = xgp.tile([EMBED, group * SUB], F32)
            nc.sync.dma_start(xg[:], x_d[:, bass.ts(g, group * SUB)])
            res_go = rgo.tile([EMBED, group * SUB], F32)
            stage = stp.tile([128, group * 2 * SPAR], F32)

            # per-subtile state tiles
            resT = [None] * group   # (128, 64) T-layout residual
            lhs = [None] * group    # (64, 128) X-layout residual (matmul lhsT)

            # t=0 lhsT comes straight from xg slices; build resT0 = X^T
            for s in range(group):
                tpx = tps.tile([128, EMBED], F32, tag="tp")
                nc.tensor.transpose(out=tpx[:], in_=xg[:, bass.ts(s, SUB)],
                                    identity=i64[:])
                rT0 = rTp.tile([128, EMBED], F32, tag="rT")
                nc.scalar.copy(rT0[:], tpx[:])
                resT[s] = rT0

            for t in range(SPAR):
                idxg = ixp.tile([128, group], mybir.dt.int32)
                for s in range(group):
                    lhsT = xg[:, bass.ts(s, SUB)] if t == 0 else lhs[s][:]
                    cp = cps.tile([SUB, NATOMS], F32)
                    nc.tensor.matmul(out=cp[:], lhsT=lhsT, rhs=dn_sb[:],
                                     start=True, stop=True)
                    c2 = c2p.tile([128, NATOMS], F32)
                    nc.scalar.square(c2[:], cp[:])
                    m2 = tiny.tile([128, 1], F32, tag="m2")
                    nc.vector.reduce_max(m2[:], c2[:], axis=AX)
                    numsl = stage[:, s * 2 * SPAR + SPAR + t:s * 2 * SPAR + SPAR + t + 1]
                    w = wp.tile([128, NATOMS], F32, tag="w")
                    nc.vector.scalar_tensor_tensor(
                        out=w[:], in0=c2[:], scalar=m2[:, 0:1], in1=cp[:],
                        op0=OP.is_equal, op1=OP.mult, accum_out=numsl)
                    idxsl = stage[:, s * 2 * SPAR + t:s * 2 * SPAR + t + 1]
                    w2 = wp.tile([128, NATOMS], F32, tag="w")
                    nc.vector.scalar_tensor_tensor(
                        out=w2[:], in0=c2[:], scalar=m2[:, 0:1], in1=cn_sb[:],
                        op0=OP.is_equal, op1=OP.mult, accum_out=idxsl)
                    nc.vector.tensor_copy(idxg[:, s:s + 1], idxsl)

                dsel = dsp.tile([128, group * (EMBED + 1)], F32)
                for s in range(group):
                    nc.gpsimd.indirect_dma_start(
                        out=dsel[:, s * (EMBED + 1):(s + 1) * (EMBED + 1)],
                        out_offset=None, in_=tab_d[:],
                        in_offset=bass.IndirectOffsetOnAxis(ap=idxg[:, s:s + 1], axis=0),
                        bounds_check=NATOMS - 1, oob_is_err=False)

                for s in range(group):
                    base = s * (EMBED + 1)
                    numsl = stage[:, s * 2 * SPAR + SPAR + t:s * 2 * SPAR + SPAR + t + 1]
                    aneg = tiny.tile([128, 1], F32, tag="aneg")
                    # tab col EMBED holds -1/(s[n]+eps)  ->  aneg = -alpha
                    nc.vector.tensor_tensor(out=aneg[:], in0=numsl,
                                            in1=dsel[:, base + EMBED:base + EMBED + 1],
                                            op=OP.mult)
                    rT_new = rTp.tile([128, EMBED], F32, tag="rT")
                    nc.vector.scalar_tensor_tensor(
                        out=rT_new[:], in0=dsel[:, base:base + EMBED],
                        scalar=aneg[:, 0:1], in1=resT[s][:],
                        op0=OP.mult, op1=OP.add)
                    resT[s] = rT_new
                    # transpose back to X-layout: next lhsT, or final output
                    tpr = tps.tile([EMBED, SUB], F32, tag="tp")
                    nc.tensor.transpose(out=tpr[:], in_=rT_new[:], identity=i128[:])
                    if t < SPAR - 1:
                        nl = rsp.tile([EMBED, SUB], F32, tag="rs")
                        nc.scalar.copy(nl[:], tpr[:])
                        lhs[s] = nl
                    else:
                        nc.scalar.copy(res_go[:, bass.ts(s, SUB)], tpr[:])

            nc.sync.dma_start(res_d[:, bass.ts(g, group * SUB)], res_go[:])
            nc.sync.dma_start(sc_d[bass.ts(g, 128), :], stage[:])

    nc.compile()
    return nc


def host_prepare(z_e: np.ndarray, dictionary: np.ndarray):
    z_p = np.transpose(z_e, (0, 2, 3, 1))          # (B,H,W,C)
    X = np.ascontiguousarray(z_p).reshape(EMBED, -1)
    norms = np.sqrt((dictionary.astype(np.float32) ** 2).sum(axis=0,
                    dtype=np.float32)).astype(np.float32)
    Dn = (dictionary / norms).astype(np.float32)
    s = (Dn * Dn).sum(axis=0, dtype=np.float32).astype(np.float32)
    negrecip = (np.float32(-1.0) / (s + np.float32(EPS))).astype(np.float32)
    tab = np.concatenate([np.ascontiguousarray(Dn.T), negrecip[:, None]],
                         axis=1).astype(np.float32)  # (512, 65)
    cn = np.broadcast_to(np.arange(NATOMS, dtype=np.float32), (128, NATOMS)).copy()
    return X, Dn, s, tab, cn


def host_finalize(X, s, res_full, idx_all, num_all, B_shape):
    """res_full (64, N); idx_all/num_all (SPAR, N)."""
    N = X.shape[1]
    z_flat = X - res_full
    z_out = z_flat.reshape(B_shape[0], B_shape[2], B_shape[3], B_shape[1])
    z_out = np.transpose(z_out, (0, 3, 1, 2)).copy()

    loss = np.float32(1.25 * np.mean(res_full.astype(np.float64) ** 2))

    idx = idx_all.astype(np.int64)
    np.clip(idx, 0, NATOMS - 1, out=idx)
    alpha = (num_all / (s[idx] + np.float32(EPS))).astype(np.float32)
    coeffs = np.zeros((NATOMS, N), dtype=np.float32)
    cols = np.broadcast_to(np.arange(N), (SPAR, N))
    srt = np.sort(idx, axis=0)
    dupcols = (srt[:-1] == srt[1:]).any(axis=0)
    if dupcols.any():
        nd = ~dupcols
        coeffs[idx[:, nd], cols[:, nd]] = alpha[:, nd]
        np.add.at(coeffs, (idx[:, dupcols].ravel(), cols[:, dupcols].ravel()),
                  alpha[:, dupcols].ravel())
    else:
        coeffs[idx, cols] = alpha
    return z_out, loss, coeffs


_NC_CACHE = {}


def get_nc(cols_per_core: int, group: int):
    key = (cols_per_core, group)
    if key not in _NC_CACHE:
        _NC_CACHE[key] = build_nc(cols_per_core, group)
    return _NC_CACHE[key]


def decode_scan(sc, ngroups, group):
    """sc (ngroups*128, group*10) -> idx (SPAR, cols), num (SPAR, cols)."""
    blk = sc.reshape(ngroups, 128, group, 2 * SPAR)
    # column ordering: global col within core = (g*group + s)*128 + p
    blk = blk.transpose(0, 2, 1, 3)                  # (g, s, p, 10)
    blk = blk.reshape(ngroups * group * 128, 2 * SPAR)
    idx = blk[:, :SPAR].T                            # (SPAR, cols)
    num = blk[:, SPAR:].T
    return idx, num


def kernel(z_e: np.ndarray, dictionary: np.ndarray, _group: int = 16,
           _run=None):
    z_e = np.asarray(z_e, dtype=np.float32)
    dictionary = np.asarray(dictionary, dtype=np.float32)
    X, Dn, s, tab, cn = host_prepare(z_e, dictionary)
    N = X.shape[1]
    cols_per_core = N // NCORES
    nsub = cols_per_core // SUB
    ngroups = nsub // _group

    nc = get_nc(cols_per_core, _group)
    in_maps = []
    for c in range(NCORES):
        in_maps.append({
            "x": np.ascontiguousarray(X[:, c * cols_per_core:(c + 1) * cols_per_core]),
            "dn": Dn, "constn": cn, "tab": tab,
        })
    if _run is None:
        results = run_bass_kernel_spmd(nc, in_maps, core_ids=list(range(NCORES))).results
    else:
        results = _run(nc, in_maps)

    res_full = np.concatenate([results[c]["res"] for c in range(NCORES)], axis=1)
    idx_parts, num_parts = [], []
    for c in range(NCORES):
        idx, num = decode_scan(results[c]["scan"], ngroups, _group)
        idx_parts.append(idx)
        num_parts.append(num)
    idx_all = np.concatenate(idx_parts, axis=1)
    num_all = np.concatenate(num_parts, axis=1)

    return host_finalize(X, s, res_full, idx_all, num_all, z_e.shape)


# revision 8
# speedup vs baseline: 1.3617x; 1.3617x over previous
"""Trainium2 Bass kernel for nn_DictionaryLearning (batch OMP / vq_codebook).

Strategy (data-parallel over the flattened sample axis, per sharding hint):
- Host: z_e (B,C,H,W) -> channels-last -> raw reshape X (64, 131072).
  Shard columns across 8 cores (16384 each). Dictionary replicated.
- Device per core: for each 128-column subtile run 5 OMP iterations:
    corr   = matmul(lhsT=residual(64,128), rhs=Dn(64,512)) -> PSUM (128 cols, 512 atoms)
    corr2  = square(corr)                 [ScalarE, PSUM->SBUF]
    m2     = reduce_max(corr2)            [VectorE]
    num    = accum[(corr2==m2)*corr]      [VectorE scalar_tensor_tensor, signed winner]
    idx    = accum[(corr2==m2)*iota]      [VectorE scalar_tensor_tensor]
    d_sel  = indirect-DMA gather of [DnT | -s] rows by idx (batched per group)
    -alpha = num / (-s[idx])              [VectorE divide]
    resT  -= alpha*d_selT                 [VectorE fused mult-add, T-layout]
    res    = transpose(resT)              [TensorE + ScalarE evac] for next matmul
- Outputs: final residual (64, cols) + per-subtile (idx, num) pairs.
  Host reconstructs z_out = X - residual, loss = 1.25*mean(res^2),
  coeffs scattered sparse -> dense with alpha = num/(s[idx]+eps).
"""
import sys
for _p in ('/opt/trn_rl_repo', '/root/.axon_site/_ro/trn_rl_repo'):
    if _p not in sys.path:
        sys.path.insert(0, _p)
from contextlib import ExitStack

import numpy as np

import concourse.bass as bass
import concourse.bacc as bacc
import concourse.tile as tile
from concourse import mybir
from concourse.bass_utils import run_bass_kernel_spmd
from concourse.masks import make_identity

F32 = mybir.dt.float32
EMBED = 64
NATOMS = 512
SPAR = 5
NCORES = 8
TOTAL_COLS = 32 * 64 * 64  # 131072
SUB = 128                  # columns per subtile (matmul out partitions)
EPS = 1e-10


def build_nc(cols_per_core: int, group: int):
    """Build + compile the per-core SPMD bass module."""
    nsub = cols_per_core // SUB
    ngroups = nsub // group
    assert nsub * SUB == cols_per_core and ngroups * group == nsub

    nc = bacc.Bacc("TRN2", target_bir_lowering=False, debug=False)

    x_d = nc.dram_tensor("x", [EMBED, cols_per_core], F32, kind="ExternalInput")
    dn_d = nc.dram_tensor("dn", [EMBED, NATOMS], F32, kind="ExternalInput")
    cn_d = nc.dram_tensor("constn", [128, NATOMS], F32, kind="ExternalInput")
    tab_d = nc.dram_tensor("tab", [NATOMS, EMBED + 1], F32, kind="ExternalInput")

    res_d = nc.dram_tensor("res", [EMBED, cols_per_core], F32, kind="ExternalOutput")
    sc_d = nc.dram_tensor("scan", [ngroups * 128, group * 2 * SPAR], F32,
                          kind="ExternalOutput")

    AX = mybir.AxisListType.X
    OP = mybir.AluOpType

    with tile.TileContext(nc) as tc, ExitStack() as ctx:
        const = ctx.enter_context(tc.tile_pool(name="const", bufs=1))
        xgp = ctx.enter_context(tc.tile_pool(name="xg", bufs=2))
        rgo = ctx.enter_context(tc.tile_pool(name="resgo", bufs=2))
        c2p = ctx.enter_context(tc.tile_pool(name="c2", bufs=3))
        wp = ctx.enter_context(tc.tile_pool(name="w", bufs=3))
        rTp = ctx.enter_context(tc.tile_pool(name="rT", bufs=2 * group + 2))
        rsp = ctx.enter_context(tc.tile_pool(name="rs", bufs=group + 2))
        dsp = ctx.enter_context(tc.tile_pool(name="dsel", bufs=2))
        stp = ctx.enter_context(tc.tile_pool(name="stage", bufs=2))
        ixp = ctx.enter_context(tc.tile_pool(name="idxg", bufs=2))
        tiny = ctx.enter_context(tc.tile_pool(name="tiny", bufs=12))
        cps = ctx.enter_context(tc.tile_pool(name="cpsum", bufs=3, space="PSUM"))
        tps = ctx.enter_context(tc.tile_pool(name="tpsum", bufs=4, space="PSUM"))

        npair = group // 2
        dn2 = const.tile([128, NATOMS], F32)          # Dn duplicated in both halves
        nc.sync.dma_start(dn2[0:EMBED, :], dn_d[:])
        nc.sync.dma_start(dn2[EMBED:128, :], dn_d[:])
        cn_sb = const.tile([128, NATOMS], F32)
        nc.sync.dma_start(cn_sb[:], cn_d[:])
        i128 = const.tile([128, 128], F32)
        make_identity(nc, i128[:])

        for g in range(ngroups):
            # paired X layout: top half = even subtiles, bottom = odd
            xg = xgp.tile([128, npair * SUB], F32)
            xv = x_d[:, bass.ts(g, group * SUB)].rearrange(
                "p (i k n) -> p i k n", k=2, n=SUB)
            xgv = xg[:].rearrange("p (i n) -> p i n", n=SUB)
            nc.sync.dma_start(xgv[0:EMBED], xv[:, :, 0, :])
            nc.sync.dma_start(xgv[EMBED:128], xv[:, :, 1, :])

            res_go = rgo.tile([128, npair * SUB], F32)
            stage = stp.tile([128, group * 2 * SPAR], F32)

            resT = [None] * npair   # (128, 128): [:, 0:64]=even subtile, [:, 64:128]=odd
            lhs = [None] * npair    # (128, 128): [0:64]=even subtile res, [64:128]=odd

            # resT0 = X^T: one paired transpose per pair
            for sp in range(npair):
                tpx = tps.tile([128, 128], F32, tag="tp")
                nc.tensor.transpose(out=tpx[:], in_=xg[:, bass.ts(sp, SUB)],
                                    identity=i128[:])
                rT0 = rTp.tile([128, 128], F32, tag="rT")
                nc.scalar.copy(rT0[:], tpx[:])
                resT[sp] = rT0

            for t in range(SPAR):
                idxg = ixp.tile([128, group], mybir.dt.int32)
                for sp in range(npair):
                    cpair = [None, None]
                    c2 = c2p.tile([128, 2 * NATOMS], F32)
                    for k in range(2):
                        lhsT = (xg[:, bass.ts(sp, SUB)] if t == 0 else lhs[sp][:])
                        half = lhsT[k * EMBED:(k + 1) * EMBED, :]
                        cp = cps.tile([SUB, NATOMS], F32)
                        nc.tensor.matmul(out=cp[:], lhsT=half,
                                         rhs=dn2[k * EMBED:(k + 1) * EMBED, :],
                                         start=True, stop=True)
                        cpair[k] = cp
                        nc.scalar.square(c2[:, bass.ts(k, NATOMS)], cp[:])
                    m2p = tiny.tile([128, 2], F32, tag="m2")
                    nc.vector.reduce_max(
                        m2p[:], c2[:].rearrange("p (k n) -> p k n", k=2), axis=AX)
                    for k in range(2):
                        s = 2 * sp + k
                        cp = cpair[k]
                        c2k = c2[:, bass.ts(k, NATOMS)]
                        numsl = stage[:, s * 2 * SPAR + SPAR + t:s * 2 * SPAR + SPAR + t + 1]
                        w = wp.tile([128, NATOMS], F32, tag="w")
                        nc.vector.scalar_tensor_tensor(
                            out=w[:], in0=c2k, scalar=m2p[:, k:k + 1], in1=cp[:],
                            op0=OP.is_equal, op1=OP.mult, accum_out=numsl)
                        idxsl = stage[:, s * 2 * SPAR + t:s * 2 * SPAR + t + 1]
                        w2 = wp.tile([128, NATOMS], F32, tag="w")
                        nc.vector.scalar_tensor_tensor(
                            out=w2[:], in0=c2k, scalar=m2p[:, k:k + 1], in1=cn_sb[:],
                            op0=OP.is_equal, op1=OP.mult, accum_out=idxsl)
                        nc.scalar.copy(idxg[:, s:s + 1], idxsl)

                dsel = dsp.tile([128, group * (EMBED + 1)], F32)
                for s in range(group):
                    nc.gpsimd.indirect_dma_start(
                        out=dsel[:, s * (EMBED + 1):(s + 1) * (EMBED + 1)],
                        out_offset=None, in_=tab_d[:],
                        in_offset=bass.IndirectOffsetOnAxis(ap=idxg[:, s:s + 1], axis=0),
                        bounds_check=NATOMS - 1, oob_is_err=False)

                for sp in range(npair):
                    rT_new = rTp.tile([128, 128], F32, tag="rT")
                    for k in range(2):
                        s = 2 * sp + k
                        base = s * (EMBED + 1)
                        numsl = stage[:, s * 2 * SPAR + SPAR + t:s * 2 * SPAR + SPAR + t + 1]
                        aneg = tiny.tile([128, 1], F32, tag="aneg")
                        # tab col EMBED holds -1/(s[n]+eps) -> aneg = -alpha (ACT)
                        nc.scalar.activation(aneg[:], numsl,
                                             mybir.ActivationFunctionType.Copy,
                                             scale=dsel[:, base + EMBED:base + EMBED + 1])
                        nc.vector.scalar_tensor_tensor(
                            out=rT_new[:, k * EMBED:(k + 1) * EMBED],
                            in0=dsel[:, base:base + EMBED],
                            scalar=aneg[:, 0:1],
                            in1=resT[sp][:, k * EMBED:(k + 1) * EMBED],
                            op0=OP.mult, op1=OP.add)
                    resT[sp] = rT_new
                    # one paired transpose back to X-layout halves
                    tpr = tps.tile([128, 128], F32, tag="tp")
                    nc.tensor.transpose(out=tpr[:], in_=rT_new[:], identity=i128[:])
                    if t < SPAR - 1:
                        nl = rsp.tile([128, 128], F32, tag="rs")
                        nc.scalar.copy(nl[:], tpr[:])
                        lhs[sp] = nl
                    else:
                        nc.scalar.copy(res_go[:, bass.ts(sp, SUB)], tpr[:])

            # outputs: even subtiles from top half, odd from bottom half
            rgv = res_go[:].rearrange("p (i n) -> p i n", n=SUB)
            ov = res_d[:, bass.ts(g, group * SUB)].rearrange(
                "p (i k n) -> p i k n", k=2, n=SUB)
            nc.sync.dma_start(ov[:, :, 0, :], rgv[0:EMBED])
            nc.sync.dma_start(ov[:, :, 1, :], rgv[EMBED:128])
            nc.sync.dma_start(sc_d[bass.ts(g, 128), :], stage[:])

    nc.compile()
    return nc


def host_prepare(z_e: np.ndarray, dictionary: np.ndarray):
    z_p = np.transpose(z_e, (0, 2, 3, 1))          # (B,H,W,C)
    X = np.ascontiguousarray(z_p).reshape(EMBED, -1)
    norms = np.sqrt((dictionary.astype(np.float32) ** 2).sum(axis=0,
                    dtype=np.float32)).astype(np.float32)
    Dn = (dictionary / norms).astype(np.float32)
    s = (Dn * Dn).sum(axis=0, dtype=np.float32).astype(np.float32)
    negrecip = (np.float32(-1.0) / (s + np.float32(EPS))).astype(np.float32)
    tab = np.concatenate([np.ascontiguousarray(Dn.T), negrecip[:, None]],
                         axis=1).astype(np.float32)  # (512, 65)
    cn = np.broadcast_to(np.arange(NATOMS, dtype=np.float32), (128, NATOMS)).copy()
    return X, Dn, s, tab, cn


def host_finalize(X, s, res_full, idx_all, num_all, B_shape):
    """res_full (64, N); idx_all/num_all (SPAR, N)."""
    N = X.shape[1]
    z_flat = X - res_full
    z_out = z_flat.reshape(B_shape[0], B_shape[2], B_shape[3], B_shape[1])
    z_out = np.transpose(z_out, (0, 3, 1, 2)).copy()

    loss = np.float32(1.25 * np.mean(res_full.astype(np.float64) ** 2))

    idx = idx_all.astype(np.int64)
    np.clip(idx, 0, NATOMS - 1, out=idx)
    alpha = (num_all / (s[idx] + np.float32(EPS))).astype(np.float32)
    coeffs = np.zeros((NATOMS, N), dtype=np.float32)
    cols = np.broadcast_to(np.arange(N), (SPAR, N))
    srt = np.sort(idx, axis=0)
    dupcols = (srt[:-1] == srt[1:]).any(axis=0)
    if dupcols.any():
        nd = ~dupcols
        coeffs[idx[:, nd], cols[:, nd]] = alpha[:, nd]
        np.add.at(coeffs, (idx[:, dupcols].ravel(), cols[:, dupcols].ravel()),
                  alpha[:, dupcols].ravel())
    else:
        coeffs[idx, cols] = alpha
    return z_out, loss, coeffs


_NC_CACHE = {}


def get_nc(cols_per_core: int, group: int):
    key = (cols_per_core, group)
    if key not in _NC_CACHE:
        _NC_CACHE[key] = build_nc(cols_per_core, group)
    return _NC_CACHE[key]


def decode_scan(sc, ngroups, group):
    """sc (ngroups*128, group*10) -> idx (SPAR, cols), num (SPAR, cols)."""
    blk = sc.reshape(ngroups, 128, group, 2 * SPAR)
    # column ordering: global col within core = (g*group + s)*128 + p
    blk = blk.transpose(0, 2, 1, 3)                  # (g, s, p, 10)
    blk = blk.reshape(ngroups * group * 128, 2 * SPAR)
    idx = blk[:, :SPAR].T                            # (SPAR, cols)
    num = blk[:, SPAR:].T
    return idx, num


def kernel(z_e: np.ndarray, dictionary: np.ndarray, _group: int = 16,
           _run=None):
    z_e = np.asarray(z_e, dtype=np.float32)
    dictionary = np.asarray(dictionary, dtype=np.float32)
    X, Dn, s, tab, cn = host_prepare(z_e, dictionary)
    N = X.shape[1]
    cols_per_core = N // NCORES
    nsub = cols_per_core // SUB
    ngroups = nsub // _group

    nc = get_nc(cols_per_core, _group)
    in_maps = []
    for c in range(NCORES):
        in_maps.append({
            "x": np.ascontiguousarray(X[:, c * cols_per_core:(c + 1) * cols_per_core]),
            "dn": Dn, "constn": cn, "tab": tab,
        })
    if _run is None:
        results = run_bass_kernel_spmd(nc, in_maps, core_ids=list(range(NCORES))).results
    else:
        results = _run(nc, in_maps)

    res_full = np.concatenate([results[c]["res"] for c in range(NCORES)], axis=1)
    idx_parts, num_parts = [], []
    for c in range(NCORES):
        idx, num = decode_scan(results[c]["scan"], ngroups, _group)
        idx_parts.append(idx)
        num_parts.append(num)
    idx_all = np.concatenate(idx_parts, axis=1)
    num_all = np.concatenate(num_parts, axis=1)

    return host_finalize(X, s, res_full, idx_all, num_all, z_e.shape)


# revision 10
# speedup vs baseline: 1.3668x; 1.0038x over previous
"""Trainium2 Bass kernel for nn_DictionaryLearning (batch OMP / vq_codebook).

Strategy (data-parallel over the flattened sample axis, per sharding hint):
- Host: z_e (B,C,H,W) -> channels-last -> raw reshape X (64, 131072).
  Shard columns across 8 cores (16384 each). Dictionary replicated.
- Device per core: for each 128-column subtile run 5 OMP iterations:
    corr   = matmul(lhsT=residual-half(64,128), rhs=Dn-half) -> PSUM (128 cols, 512 atoms)
             (subtile PAIRS share the 128x128 PE array via row groups 0-63/64-127)
    corr2  = square(corr)                 [ScalarE, PSUM->SBUF, pair-shared c2 tile]
    m2     = reduce_max(corr2)            [VectorE, 3D-batched over the pair]
    num    = accum[(corr2==m2)*corr]      [VectorE scalar_tensor_tensor, signed winner]
    idx    = accum[(corr2==m2)*iota]      [VectorE scalar_tensor_tensor]
    d_sel  = indirect-DMA gather of tab rows by idx, tab[n] = -DnT[n]/(s[n]+eps)
    resT  += num * d_sel                  [VectorE fused mult-add, T-layout == -alpha*d]
    res    = transpose(resT)              [TensorE paired 128x128 + ScalarE evac]
- Outputs: final residual (64, cols) + per-subtile (idx, num) pairs.
  Host reconstructs z_out = X - residual, loss = 1.25*mean(res^2),
  coeffs scattered sparse -> dense with alpha = num/(s[idx]+eps).
Measured (axon, 8 NC-v3): cost-model 1.26 ms; pipelined wall ~2.3 ms/call of
which ~0.9 ms is PJRT/axon dispatch floor -> kernel ~1.4 ms.
"""
import sys
for _p in ('/opt/trn_rl_repo', '/root/.axon_site/_ro/trn_rl_repo'):
    if _p not in sys.path:
        sys.path.insert(0, _p)
from contextlib import ExitStack

import numpy as np

import concourse.bass as bass
import concourse.bacc as bacc
import concourse.tile as tile
from concourse import mybir
from concourse.bass_utils import run_bass_kernel_spmd
from concourse.masks import make_identity

F32 = mybir.dt.float32
EMBED = 64
NATOMS = 512
SPAR = 5
NCORES = 8
TOTAL_COLS = 32 * 64 * 64  # 131072
SUB = 128                  # columns per subtile (matmul out partitions)
EPS = 1e-10


def build_nc(cols_per_core: int, group: int):
    """Build + compile the per-core SPMD bass module."""
    nsub = cols_per_core // SUB
    ngroups = nsub // group
    assert nsub * SUB == cols_per_core and ngroups * group == nsub

    nc = bacc.Bacc("TRN2", target_bir_lowering=False, debug=False)

    x_d = nc.dram_tensor("x", [EMBED, cols_per_core], F32, kind="ExternalInput")
    dn_d = nc.dram_tensor("dn", [EMBED, NATOMS], F32, kind="ExternalInput")
    cn_d = nc.dram_tensor("constn", [128, NATOMS], F32, kind="ExternalInput")
    tab_d = nc.dram_tensor("tab", [NATOMS, EMBED], F32, kind="ExternalInput")

    res_d = nc.dram_tensor("res", [EMBED, cols_per_core], F32, kind="ExternalOutput")
    sc_d = nc.dram_tensor("scan", [ngroups * 128, group * 2 * SPAR], F32,
                          kind="ExternalOutput")

    AX = mybir.AxisListType.X
    OP = mybir.AluOpType

    with tile.TileContext(nc) as tc, ExitStack() as ctx:
        const = ctx.enter_context(tc.tile_pool(name="const", bufs=1))
        xgp = ctx.enter_context(tc.tile_pool(name="xg", bufs=2))
        rgo = ctx.enter_context(tc.tile_pool(name="resgo", bufs=2))
        c2p = ctx.enter_context(tc.tile_pool(name="c2", bufs=3))
        wp = ctx.enter_context(tc.tile_pool(name="w", bufs=3))
        rTp = ctx.enter_context(tc.tile_pool(name="rT", bufs=2 * group + 2))
        rsp = ctx.enter_context(tc.tile_pool(name="rs", bufs=group + 2))
        dsp = ctx.enter_context(tc.tile_pool(name="dsel", bufs=2))
        stp = ctx.enter_context(tc.tile_pool(name="stage", bufs=2))
        ixp = ctx.enter_context(tc.tile_pool(name="idxg", bufs=2))
        tiny = ctx.enter_context(tc.tile_pool(name="tiny", bufs=12))
        cps = ctx.enter_context(tc.tile_pool(name="cpsum", bufs=3, space="PSUM"))
        tps = ctx.enter_context(tc.tile_pool(name="tpsum", bufs=4, space="PSUM"))

        npair = group // 2
        dn2 = const.tile([128, NATOMS], F32)          # Dn duplicated in both halves
        nc.sync.dma_start(dn2[0:EMBED, :], dn_d[:])
        nc.sync.dma_start(dn2[EMBED:128, :], dn_d[:])
        cn_sb = const.tile([128, NATOMS], F32)
        nc.sync.dma_start(cn_sb[:], cn_d[:])
        i128 = const.tile([128, 128], F32)
        make_identity(nc, i128[:])

        for g in range(ngroups):
            # paired X layout: top half = even subtiles, bottom = odd
            xg = xgp.tile([128, npair * SUB], F32)
            xv = x_d[:, bass.ts(g, group * SUB)].rearrange(
                "p (i k n) -> p i k n", k=2, n=SUB)
            xgv = xg[:].rearrange("p (i n) -> p i n", n=SUB)
            nc.sync.dma_start(xgv[0:EMBED], xv[:, :, 0, :])
            nc.sync.dma_start(xgv[EMBED:128], xv[:, :, 1, :])

            res_go = rgo.tile([128, npair * SUB], F32)
            stage = stp.tile([128, group * 2 * SPAR], F32)

            resT = [None] * npair   # (128, 128): [:, 0:64]=even subtile, [:, 64:128]=odd
            lhs = [None] * npair    # (128, 128): [0:64]=even subtile res, [64:128]=odd

            # resT0 = X^T: one paired transpose per pair
            for sp in range(npair):
                tpx = tps.tile([128, 128], F32, tag="tp")
                nc.tensor.transpose(out=tpx[:], in_=xg[:, bass.ts(sp, SUB)],
                                    identity=i128[:])
                rT0 = rTp.tile([128, 128], F32, tag="rT")
                nc.scalar.copy(rT0[:], tpx[:])
                resT[sp] = rT0

            for t in range(SPAR):
                idxg = ixp.tile([128, group], mybir.dt.int32)
                for sp in range(npair):
                    cpair = [None, None]
                    c2 = c2p.tile([128, 2 * NATOMS], F32)
                    for k in range(2):
                        lhsT = (xg[:, bass.ts(sp, SUB)] if t == 0 else lhs[sp][:])
                        half = lhsT[k * EMBED:(k + 1) * EMBED, :]
                        cp = cps.tile([SUB, NATOMS], F32)
                        nc.tensor.matmul(out=cp[:], lhsT=half,
                                         rhs=dn2[k * EMBED:(k + 1) * EMBED, :],
                                         start=True, stop=True)
                        cpair[k] = cp
                        nc.scalar.square(c2[:, bass.ts(k, NATOMS)], cp[:])
                    m2p = tiny.tile([128, 2], F32, tag="m2")
                    nc.vector.reduce_max(
                        m2p[:], c2[:].rearrange("p (k n) -> p k n", k=2), axis=AX)
                    for k in range(2):
                        s = 2 * sp + k
                        cp = cpair[k]
                        c2k = c2[:, bass.ts(k, NATOMS)]
                        numsl = stage[:, s * 2 * SPAR + SPAR + t:s * 2 * SPAR + SPAR + t + 1]
                        w = wp.tile([128, NATOMS], F32, tag="w")
                        nc.vector.scalar_tensor_tensor(
                            out=w[:], in0=c2k, scalar=m2p[:, k:k + 1], in1=cp[:],
                            op0=OP.is_equal, op1=OP.mult, accum_out=numsl)
                        idxsl = stage[:, s * 2 * SPAR + t:s * 2 * SPAR + t + 1]
                        w2 = wp.tile([128, NATOMS], F32, tag="w")
                        nc.vector.scalar_tensor_tensor(
                            out=w2[:], in0=c2k, scalar=m2p[:, k:k + 1], in1=cn_sb[:],
                            op0=OP.is_equal, op1=OP.mult, accum_out=idxsl)
                        nc.scalar.copy(idxg[:, s:s + 1], idxsl)

                dsel = dsp.tile([128, group * EMBED], F32)
                for s in range(group):
                    nc.gpsimd.indirect_dma_start(
                        out=dsel[:, s * EMBED:(s + 1) * EMBED],
                        out_offset=None, in_=tab_d[:],
                        in_offset=bass.IndirectOffsetOnAxis(ap=idxg[:, s:s + 1], axis=0),
                        bounds_check=NATOMS - 1, oob_is_err=False)

                for sp in range(npair):
                    rT_new = rTp.tile([128, 128], F32, tag="rT")
                    for k in range(2):
                        s = 2 * sp + k
                        base = s * EMBED
                        numsl = stage[:, s * 2 * SPAR + SPAR + t:s * 2 * SPAR + SPAR + t + 1]
                        # tab rows are pre-scaled by -1/(s+eps): resT += num * tab[idx]
                        nc.vector.scalar_tensor_tensor(
                            out=rT_new[:, k * EMBED:(k + 1) * EMBED],
                            in0=dsel[:, base:base + EMBED],
                            scalar=numsl,
                            in1=resT[sp][:, k * EMBED:(k + 1) * EMBED],
                            op0=OP.mult, op1=OP.add)
                    resT[sp] = rT_new
                    # one paired transpose back to X-layout halves
                    tpr = tps.tile([128, 128], F32, tag="tp")
                    nc.tensor.transpose(out=tpr[:], in_=rT_new[:], identity=i128[:])
                    if t < SPAR - 1:
                        nl = rsp.tile([128, 128], F32, tag="rs")
                        nc.scalar.copy(nl[:], tpr[:])
                        lhs[sp] = nl
                    else:
                        nc.scalar.copy(res_go[:, bass.ts(sp, SUB)], tpr[:])

            # outputs: even subtiles from top half, odd from bottom half
            rgv = res_go[:].rearrange("p (i n) -> p i n", n=SUB)
            ov = res_d[:, bass.ts(g, group * SUB)].rearrange(
                "p (i k n) -> p i k n", k=2, n=SUB)
            nc.sync.dma_start(ov[:, :, 0, :], rgv[0:EMBED])
            nc.sync.dma_start(ov[:, :, 1, :], rgv[EMBED:128])
            nc.sync.dma_start(sc_d[bass.ts(g, 128), :], stage[:])

    nc.compile()
    return nc


def host_prepare(z_e: np.ndarray, dictionary: np.ndarray):
    z_p = np.transpose(z_e, (0, 2, 3, 1))          # (B,H,W,C)
    X = np.ascontiguousarray(z_p).reshape(EMBED, -1)
    norms = np.sqrt((dictionary.astype(np.float32) ** 2).sum(axis=0,
                    dtype=np.float32)).astype(np.float32)
    Dn = (dictionary / norms).astype(np.float32)
    s = (Dn * Dn).sum(axis=0, dtype=np.float32).astype(np.float32)
    negrecip = (np.float32(-1.0) / (s + np.float32(EPS))).astype(np.float32)
    tab = (np.ascontiguousarray(Dn.T) * negrecip[:, None]).astype(np.float32)  # (512,64) = -d/(s+eps)
    cn = np.broadcast_to(np.arange(NATOMS, dtype=np.float32), (128, NATOMS)).copy()
    return X, Dn, s, tab, cn


def host_finalize(X, s, res_full, idx_all, num_all, B_shape):
    """res_full (64, N); idx_all/num_all (SPAR, N)."""
    N = X.shape[1]
    z_flat = X - res_full
    z_out = z_flat.reshape(B_shape[0], B_shape[2], B_shape[3], B_shape[1])
    z_out = np.transpose(z_out, (0, 3, 1, 2)).copy()

    loss = np.float32(1.25 * np.mean(res_full.astype(np.float64) ** 2))

    idx = idx_all.astype(np.int64)
    np.clip(idx, 0, NATOMS - 1, out=idx)
    alpha = (num_all / (s[idx] + np.float32(EPS))).astype(np.float32)
    coeffs = np.zeros((NATOMS, N), dtype=np.float32)
    cols = np.broadcast_to(np.arange(N), (SPAR, N))
    srt = np.sort(idx, axis=0)
    dupcols = (srt[:-1] == srt[1:]).any(axis=0)
    if dupcols.any():
        nd = ~dupcols
        coeffs[idx[:, nd], cols[:, nd]] = alpha[:, nd]
        np.add.at(coeffs, (idx[:, dupcols].ravel(), cols[:, dupcols].ravel()),
                  alpha[:, dupcols].ravel())
    else:
        coeffs[idx, cols] = alpha
    return z_out, loss, coeffs


_NC_CACHE = {}


def get_nc(cols_per_core: int, group: int):
    key = (cols_per_core, group)
    if key not in _NC_CACHE:
        _NC_CACHE[key] = build_nc(cols_per_core, group)
    return _NC_CACHE[key]


def decode_scan(sc, ngroups, group):
    """sc (ngroups*128, group*10) -> idx (SPAR, cols), num (SPAR, cols)."""
    blk = sc.reshape(ngroups, 128, group, 2 * SPAR)
    # column ordering: global col within core = (g*group + s)*128 + p
    blk = blk.transpose(0, 2, 1, 3)                  # (g, s, p, 10)
    blk = blk.reshape(ngroups * group * 128, 2 * SPAR)
    idx = blk[:, :SPAR].T                            # (SPAR, cols)
    num = blk[:, SPAR:].T
    return idx, num


def kernel(z_e: np.ndarray, dictionary: np.ndarray, _group: int = 16,
           _run=None):
    z_e = np.asarray(z_e, dtype=np.float32)
    dictionary = np.asarray(dictionary, dtype=np.float32)
    X, Dn, s, tab, cn = host_prepare(z_e, dictionary)
    N = X.shape[1]
    cols_per_core = N // NCORES
    nsub = cols_per_core // SUB
    ngroups = nsub // _group

    nc = get_nc(cols_per_core, _group)
    in_maps = []
    for c in range(NCORES):
        in_maps.append({
            "x": np.ascontiguousarray(X[:, c * cols_per_core:(c + 1) * cols_per_core]),
            "dn": Dn, "constn": cn, "tab": tab,
        })
    if _run is None:
        results = run_bass_kernel_spmd(nc, in_maps, core_ids=list(range(NCORES))).results
    else:
        results = _run(nc, in_maps)

    res_full = np.concatenate([results[c]["res"] for c in range(NCORES)], axis=1)
    idx_parts, num_parts = [], []
    for c in range(NCORES):
        idx, num = decode_scan(results[c]["scan"], ngroups, _group)
        idx_parts.append(idx)
        num_parts.append(num)
    idx_all = np.concatenate(idx_parts, axis=1)
    num_all = np.concatenate(num_parts, axis=1)

    return host_finalize(X, s, res_full, idx_all, num_all, z_e.shape)
